# revision 7
# baseline (speedup 1.0000x reference)
"""Trainium2 Bass kernel for nn_Decoder (Bahdanau-attention conditional-GRU decoder).

Sharding: data-parallel over batch B=64 across 8 NeuronCores (8 rows/core),
weights replicated. The sequential scan over T=48 runs fully on-device.

Per-core structure (layouts are b-major over the flattened (b, s) pair):
  prologue:  XC = xs_h @ [Wcz|Wcr|Wch] + bias   (project-first attention)
             YP = ys_e @ [Wxz|Wxr|Wxh]          (y-projections, all steps)
  loop t:    sa = s@W_sa ; PSUM: SA broadcast + uh(+b_sa) ; tanh ; W_a1-dot
             exp ; normalize ; E_mat ; gates = attend(XC)+YP[t]+s@Wh* ;
             sigmoid ; h = tanh(..+r*(s@Whh)) ; s' = s + z*(h-s)  (transposed)
  epilogue:  C.T from stored attention weights; logits = s@W_ls + y@W_ly
             + C@W_lc + bias ; maxout pairs ; DMA out.

Host-side glue: embedding gather emb[ys], per-core slicing/transposition,
final gather + ys_mask multiply.
"""
import sys
sys.path.insert(0, "/opt/trn_rl_repo")

import numpy as np

SLEN, B, TLEN = 64, 64, 48
WEMB = DEC = ALIGN = OUT = 512
ENC2 = 1024
NCORES = 8
BL = B // NCORES          # 8 batch rows per core
BS = BL * SLEN            # 512 flattened (b, s)
TG, TI = TLEN // 16, 16   # 3 groups x 16 steps

_CACHE = {}


def _build():
    import concourse.bass as bass
    import concourse.tile as tile
    from concourse import bacc, mybir

    F32 = mybir.dt.float32
    F32R = mybir.dt.float32r
    AF = mybir.ActivationFunctionType
    OP = mybir.AluOpType

    nc = bacc.Bacc("TRN2", target_bir_lowering=False, debug=False,
                   num_devices=NCORES)

    def din(name, shape):
        return nc.dram_tensor(name, shape, F32R, kind="ExternalInput").ap()

    uhT   = din("uhT",   [128, 4, BS])
    Wsa   = din("Wsa",   [128, 4, ALIGN])
    Wh    = din("Wh",    [128, 4, 1536])
    Wa1   = din("Wa1",   [128, 4])
    Ssel  = din("Ssel",  [8, BS])
    S16   = din("S16",   [128, 16, 8])
    MASK  = din("MASK",  [128, 4, 8])
    IDN   = din("IDN",   [128, 128])
    ones1 = din("ones1", [1, 128])
    sT0   = din("sT0",   [128, 32])
    ysT   = din("ysT",   [128, 4, TG, 128])
    Wx    = din("Wx",    [128, 4, 1536])
    xshT  = din("xshT",  [128, 8, BS])
    Wc0   = din("Wc0",   [128, 4, 1536])
    Wc1   = din("Wc1",   [128, 4, 1536])
    bzrh  = din("bzrh",  [1, 1536])
    xsS   = din("xsS",   [128, BL, ENC2])
    Wls   = din("Wls",   [128, 4, 2 * OUT])
    Wly   = din("Wly",   [128, 4, 2 * OUT])
    Wlc   = din("Wlc",   [128, 8, 2 * OUT])
    blg   = din("blg",   [1, 2 * OUT])

    out_d = nc.dram_tensor("out", [TLEN, BL, OUT], F32, kind="ExternalOutput").ap()

    with tile.TileContext(nc) as tc:
        with tc.tile_pool(name="pA", bufs=1) as pA, \
             tc.tile_pool(name="sbW", bufs=1) as sbW, \
             tc.tile_pool(name="pB", bufs=1) as pB:

            def load(pool, name, ap, **kw):
                t = pool.tile(list(ap.shape), ap.dtype, name="sb_" + name, **kw)
                nc.sync.dma_start(t, ap)
                return t

            def scr(shape, dtype, name):
                return sbW.tile(shape, dtype, name=name, tag="scr", bufs=12)

            s_IDN = load(pA, "IDN", IDN)
            s_ones = load(pA, "ones", ones1)
            s_ysT = load(pA, "ysT", ysT)
            s_blg = load(pA, "blg", blg)
            Est = pA.tile([128, 4, TLEN], F32R, name="Est")
            sAll = pA.tile([128, TG, 4, TI, 8], F32R, name="sAll")
            CT = pA.tile([128, 8, TG, TI, 8], F32R, name="CT")
            ID32 = s_IDN[0:1, 0:1].bitcast(F32)
            ID8 = s_IDN[0:8, 0:8].bitcast(F32)

            XC = pB.tile([128, 4, 1536], F32R, name="XC")
            YP = pB.tile([128, TG, 1536], F32R, name="YP")

            # ---- prologue: XC = xs_h @ Wc + bzrh (two passes over Wc) ----
            with tc.tile_pool(name="pC1", bufs=1) as pC1, \
                 tc.tile_pool(name="psP", bufs=2, space="PSUM") as psP:
                s_xshT = load(pC1, "xshT", xshT)
                s_bzrh = load(pC1, "bzrh", bzrh)
                s_Wc0 = load(pC1, "Wc0", Wc0)
                for m in range(4):
                    pXC = psP.tile([128, 1536], F32, name=f"pXC{m}", tag="big")
                    for nb in range(3):
                        sl = slice(nb * 512, (nb + 1) * 512)
                        for kc in range(4):
                            nc.tensor.matmul(pXC[:, sl],
                                             s_xshT[:, kc, m * 128:(m + 1) * 128],
                                             s_Wc0[:, kc, sl],
                                             start=(kc == 0), stop=False)
                        nc.tensor.matmul(pXC[:, sl], s_ones, s_bzrh[:, sl],
                                         start=False, stop=True)
                    if m % 2 == 0:
                        nc.vector.tensor_copy(XC[:, m, :], pXC)
                    else:
                        nc.scalar.copy(XC[:, m, :], pXC)
                with tc.tile_pool(name="pC1b", bufs=1) as pC1b:
                    s_Wc1 = load(pC1b, "Wc1", Wc1)
                    for m in range(4):
                        pXC = psP.tile([128, 1536], F32, name=f"pXCb{m}", tag="big")
                        for nb in range(3):
                            sl = slice(nb * 512, (nb + 1) * 512)
                            for kc in range(4):
                                nc.tensor.matmul(
                                    pXC[:, sl],
                                    s_xshT[:, kc + 4, m * 128:(m + 1) * 128],
                                    s_Wc1[:, kc, sl],
                                    start=(kc == 0), stop=(kc == 3))
                        nc.vector.tensor_tensor(XC[:, m, :],
                                                XC[:, m, :].bitcast(F32),
                                                pXC, op=OP.add)

            # ---- prologue: YP = ys_e @ Wx ----
            with tc.tile_pool(name="pC2", bufs=1) as pC2, \
                 tc.tile_pool(name="psP2", bufs=2, space="PSUM") as psP2:
                s_Wx = load(pC2, "Wx", Wx)
                for tg in range(TG):
                    pYP = psP2.tile([128, 1536], F32, name=f"pYP{tg}", tag="big")
                    for nb in range(3):
                        sl = slice(nb * 512, (nb + 1) * 512)
                        for kc in range(4):
                            nc.tensor.matmul(pYP[:, sl], s_ysT[:, kc, tg, :],
                                             s_Wx[:, kc, sl],
                                             start=(kc == 0), stop=(kc == 3))
                    if tg % 2 == 0:
                        nc.vector.tensor_copy(YP[:, tg, :], pYP)
                    else:
                        nc.scalar.copy(YP[:, tg, :], pYP)

            # ---- loop-resident tensors + the recurrent loop ----
            with tc.tile_pool(name="pB2", bufs=1) as pB2, \
                 tc.tile_pool(name="psL", bufs=1, space="PSUM") as psL:
                s_uhT = load(pB2, "uhT", uhT)
                s_Wsa = load(pB2, "Wsa", Wsa)
                s_Wh = load(pB2, "Wh", Wh)
                s_Wa1 = load(pB2, "Wa1", Wa1)
                s_Ssel = load(pB2, "Ssel", Ssel)
                s_S16 = load(pB2, "S16", S16)
                s_MASK = load(pB2, "MASK", MASK)

                sT = sbW.tile([128, 32], F32R, name="sT_init", tag="sT", bufs=2)
                nc.sync.dma_start(sT, sT0)

                for t in range(TLEN):
                    tg, ti = t // TI, t % TI
                    p_sa = psL.tile([8, 512], F32, name=f"psa{t}", tag="sa")
                    for kc in range(4):
                        nc.tensor.matmul(p_sa, sT[:, kc * 8:(kc + 1) * 8],
                                         s_Wsa[:, kc, :],
                                         start=(kc == 0), stop=(kc == 3))
                    sb_sa = scr([8, 512], F32R, f"sbsa{t}")
                    nc.scalar.copy(sb_sa, p_sa)

                    p_e = psL.tile([1, BS], F32, name=f"pe{t}", tag="pe")
                    for ac in range(4):
                        p_att = psL.tile([128, BS], F32, name=f"pat{t}_{ac}",
                                         tag="att", bufs=2)
                        nc.tensor.matmul(p_att, sb_sa[:, ac * 128:(ac + 1) * 128],
                                         s_Ssel, start=True, stop=False)
                        nc.tensor.matmul(p_att, s_IDN, s_uhT[:, ac, :],
                                         start=False, stop=True)
                        th = sbW.tile([128, BS], F32R, name=f"th{t}_{ac}",
                                      tag="th", bufs=2)
                        nc.scalar.activation(th, p_att, AF.Tanh)
                        nc.tensor.matmul(p_e, s_Wa1[:, ac:ac + 1], th,
                                         start=(ac == 0), stop=(ac == 3))

                    sb_e = scr([1, BS], F32, f"sbe{t}")
                    nc.scalar.activation(sb_e, p_e, AF.Exp)
                    sb_sum = scr([1, 8], F32, f"sbsum{t}")
                    nc.vector.tensor_reduce(
                        sb_sum, sb_e.rearrange("p (b s) -> p b s", b=8),
                        axis=mybir.AxisListType.X, op=OP.add)
                    sb_rec = scr([1, 8], F32, f"sbrec{t}")
                    nc.vector.reciprocal(sb_rec, sb_sum)
                    sb_en = scr([1, BS], F32, f"sben{t}")
                    rec_bc = bass.AP(tensor=sb_rec.tensor, offset=sb_rec.offset,
                                     ap=[sb_rec.ap[0], sb_rec.ap[1], [0, SLEN]])
                    nc.vector.tensor_tensor(
                        sb_en.rearrange("p (b s) -> p b s", b=8),
                        sb_e.rearrange("p (b s) -> p b s", b=8),
                        rec_bc, op=OP.mult)

                    p_ec = psL.tile([128, 4], F32, name=f"pec{t}", tag="sa")
                    for c in range(4):
                        nc.tensor.transpose(p_ec[:, c:c + 1],
                                            sb_en[:, c * 128:(c + 1) * 128], ID32)
                    sb_E = scr([128, 4, 8], F32R, f"sbE{t}")
                    for c in range(4):
                        nc.vector.tensor_scalar_mul(sb_E[:, c, :],
                                                    s_MASK[:, c, :],
                                                    p_ec[:, c:c + 1])
                    nc.vector.tensor_copy(Est[:, :, t], p_ec)

                    p_zr = psL.tile([8, 1024], F32, name=f"pzr{t}", tag="zr")
                    p_h = psL.tile([8, 512], F32, name=f"ph{t}", tag="h")
                    p_hh = psL.tile([8, 512], F32, name=f"phh{t}", tag="hh")
                    for nb in range(2):
                        sl = slice(nb * 512, (nb + 1) * 512)
                        for kc in range(4):
                            nc.tensor.matmul(p_zr[:, sl],
                                             sT[:, kc * 8:(kc + 1) * 8],
                                             s_Wh[:, kc, sl],
                                             start=(kc == 0), stop=False)
                        for c in range(4):
                            nc.tensor.matmul(p_zr[:, sl], sb_E[:, c, :],
                                             XC[:, c, sl], start=False, stop=False)
                        nc.tensor.matmul(p_zr[:, sl], s_S16[:, ti, :],
                                         YP[:, tg, sl], start=False, stop=True)
                    hs = slice(1024, 1536)
                    for c in range(4):
                        nc.tensor.matmul(p_h, sb_E[:, c, :], XC[:, c, hs],
                                         start=(c == 0), stop=False)
                    nc.tensor.matmul(p_h, s_S16[:, ti, :], YP[:, tg, hs],
                                     start=False, stop=True)
                    for kc in range(4):
                        nc.tensor.matmul(p_hh, sT[:, kc * 8:(kc + 1) * 8],
                                         s_Wh[:, kc, hs],
                                         start=(kc == 0), stop=(kc == 3))

                    sb_zr = scr([8, 1024], F32, f"sbzr{t}")
                    nc.scalar.activation(sb_zr, p_zr, AF.Sigmoid)
                    sb_t1 = scr([8, 512], F32, f"sbt1{t}")
                    nc.vector.tensor_tensor(sb_t1, sb_zr[:, 512:1024], p_hh,
                                            op=OP.mult)
                    sb_hp = scr([8, 512], F32, f"sbhp{t}")
                    nc.vector.tensor_tensor(sb_hp, sb_t1, p_h, op=OP.add)
                    sb_h = scr([8, 512], F32, f"sbh{t}")
                    nc.scalar.activation(sb_h, sb_hp, AF.Tanh)

                    p_zhT = psL.tile([128, 64], F32, name=f"pzhT{t}", tag="pe")
                    for c in range(4):
                        nc.tensor.transpose(p_zhT[:, c * 8:(c + 1) * 8],
                                            sb_zr[:, c * 128:(c + 1) * 128], ID8)
                        nc.tensor.transpose(p_zhT[:, 32 + c * 8:32 + (c + 1) * 8],
                                            sb_h[:, c * 128:(c + 1) * 128], ID8)

                    sb_d = scr([128, 32], F32, f"sbd{t}")
                    nc.vector.tensor_tensor(sb_d, p_zhT[:, 32:64],
                                            sT.bitcast(F32), op=OP.subtract)
                    sb_zd = scr([128, 32], F32, f"sbzd{t}")
                    nc.vector.tensor_tensor(sb_zd, p_zhT[:, 0:32], sb_d, op=OP.mult)
                    sT_new = sbW.tile([128, 32], F32R, name=f"sT{t}",
                                      tag="sT", bufs=2)
                    nc.vector.tensor_tensor(sT_new, sT.bitcast(F32), sb_zd,
                                            op=OP.add)
                    sT = sT_new
                    nc.vector.tensor_copy(sAll[:, tg, :, ti, :],
                                          sT.rearrange("p (k b) -> p k b", k=4))

            # ================= epilogue =================
            with tc.tile_pool(name="pD", bufs=1) as pD, \
                 tc.tile_pool(name="psE", bufs=1, space="PSUM") as psE:
                for b in range(BL):
                    x_b = load(pD, f"xs{b}", xsS[:, b, :], tag="xsb", bufs=2)
                    p_ct = psE.tile([128, 8, TLEN], F32, name=f"pct{b}",
                                    tag="ct", bufs=2)
                    base = (b % 2) * 64
                    eb = Est[base:base + 64, b // 2, :]
                    for dc in range(8):
                        nc.tensor.matmul(p_ct[:, dc, :],
                                         x_b[base:base + 64,
                                             dc * 128:(dc + 1) * 128],
                                         eb, start=True, stop=True)
                    nc.vector.tensor_copy(
                        CT[:, :, :, :, b],
                        p_ct.rearrange("p dc (tg ti) -> p dc tg ti", tg=TG))

                for tg in range(TG):
                    p_L = psE.tile([128, 1024], F32, name=f"pL{tg}", tag="L")
                    for nb in range(2):
                        sl = slice(nb * 512, (nb + 1) * 512)
                        w_ls = load(pD, f"wls{tg}{nb}", Wls[:, :, sl],
                                    tag="wls", bufs=2)
                        w_ly = load(pD, f"wly{tg}{nb}", Wly[:, :, sl],
                                    tag="wly", bufs=2)
                        w_lc = load(pD, f"wlc{tg}{nb}", Wlc[:, :, sl],
                                    tag="wlc", bufs=2)
                        for kc in range(4):
                            nc.tensor.matmul(
                                p_L[:, sl],
                                sAll[:, tg, kc, :, :].rearrange("p ti b -> p (ti b)"),
                                w_ls[:, kc, :], start=(kc == 0), stop=False)
                        for kc in range(4):
                            nc.tensor.matmul(p_L[:, sl], s_ysT[:, kc, tg, :],
                                             w_ly[:, kc, :],
                                             start=False, stop=False)
                        for dc in range(8):
                            nc.tensor.matmul(
                                p_L[:, sl],
                                CT[:, dc, tg, :, :].rearrange("p ti b -> p (ti b)"),
                                w_lc[:, dc, :], start=False, stop=False)
                        nc.tensor.matmul(p_L[:, sl], s_ones, s_blg[:, sl],
                                         start=False, stop=True)
                    sb_f = scr([128, 1024], F32, f"sbf{tg}")
                    nc.scalar.copy(sb_f, p_L)
                    sb_o = scr([128, 512], F32, f"sbo{tg}")
                    ap_even = bass.AP(tensor=sb_f.tensor, offset=sb_f.offset,
                                      ap=[sb_f.ap[0], [2, 512]])
                    ap_odd = bass.AP(tensor=sb_f.tensor, offset=sb_f.offset + 1,
                                     ap=[sb_f.ap[0], [2, 512]])
                    nc.vector.tensor_tensor(sb_o, ap_even, ap_odd, op=OP.max)
                    nc.sync.dma_start(
                        out_d[tg * TI:(tg + 1) * TI].rearrange("t b f -> (t b) f"),
                        sb_o)

    nc.compile()
    return nc


def _prep_core(c, s0, xs_h, uh, ys_e, W_sa, b_sa, W_a1,
               Wxz, Whz, Wcz, bz, Wxr, Whr, Wcr, br, Wxh, Whh, Wch, bh,
               W_ls, b_ls, W_ly, b_ly, W_lc, b_lc, const_cache):
    f32 = np.float32
    bsl = slice(c * BL, (c + 1) * BL)
    uh_c = uh[:, bsl, :]
    xs_c = xs_h[:, bsl, :]
    ye_c = ys_e[:, bsl, :]
    s0_c = s0[bsl, :]

    def kchunks(w, kc):
        return np.ascontiguousarray(
            w.reshape(kc, 128, w.shape[1]).transpose(1, 0, 2))

    m = {}
    u = (uh_c + b_sa[None, None, :]).transpose(2, 1, 0).reshape(512, BS)
    m["uhT"] = np.ascontiguousarray(u.reshape(4, 128, BS).transpose(1, 0, 2))
    m["Wsa"] = const_cache.setdefault("Wsa", kchunks(W_sa, 4))
    m["Wh"] = const_cache.setdefault(
        "Wh", kchunks(np.concatenate([Whz, Whr, Whh], 1), 4))
    m["Wa1"] = const_cache.setdefault(
        "Wa1", np.ascontiguousarray(W_a1.reshape(4, 128).T))
    if "Ssel" not in const_cache:
        ssel = np.zeros((8, BS), f32)
        for b in range(BL):
            ssel[b, b * SLEN:(b + 1) * SLEN] = 1.0
        s16 = np.zeros((16, 128, 8), f32)
        for ti in range(16):
            for b in range(8):
                s16[ti, ti * 8 + b, b] = 1.0
        mask = np.zeros((128, 4, 8), f32)
        for ch in range(4):
            for p in range(128):
                mask[p, ch, (ch * 128 + p) // SLEN] = 1.0
        const_cache["Ssel"] = ssel
        const_cache["S16"] = np.ascontiguousarray(s16.transpose(1, 0, 2))
        const_cache["MASK"] = mask
        const_cache["IDN"] = np.eye(128, dtype=f32)
        const_cache["ones1"] = np.ones((1, 128), f32)
    for k in ("Ssel", "S16", "MASK", "IDN", "ones1"):
        m[k] = const_cache[k]
    m["sT0"] = np.ascontiguousarray(
        s0_c.T.reshape(4, 128, 8).transpose(1, 0, 2).reshape(128, 32))
    yt = ye_c.transpose(2, 0, 1).reshape(512, TG, TI * 8)
    m["ysT"] = np.ascontiguousarray(
        yt.reshape(4, 128, TG, 128).transpose(1, 0, 2, 3))
    m["Wx"] = const_cache.setdefault(
        "Wx", kchunks(np.concatenate([Wxz, Wxr, Wxh], 1), 4))
    xt = xs_c.transpose(2, 1, 0).reshape(ENC2, BS)
    m["xshT"] = np.ascontiguousarray(xt.reshape(8, 128, BS).transpose(1, 0, 2))
    if "Wc0" not in const_cache:
        wc = kchunks(np.concatenate([Wcz, Wcr, Wch], 1), 8)
        const_cache["Wc0"] = np.ascontiguousarray(wc[:, 0:4])
        const_cache["Wc1"] = np.ascontiguousarray(wc[:, 4:8])
    m["Wc0"], m["Wc1"] = const_cache["Wc0"], const_cache["Wc1"]
    m["bzrh"] = const_cache.setdefault(
        "bzrh", np.concatenate([bz, br, bh])[None, :].astype(f32))
    m["xsS"] = np.ascontiguousarray(np.concatenate([xs_c, xs_c], axis=0))
    m["Wls"] = const_cache.setdefault("Wls", kchunks(W_ls, 4))
    m["Wly"] = const_cache.setdefault("Wly", kchunks(W_ly, 4))
    m["Wlc"] = const_cache.setdefault("Wlc", kchunks(W_lc, 8))
    m["blg"] = const_cache.setdefault(
        "blg", (b_ls + b_ly + b_lc)[None, :].astype(f32))
    return {k: np.ascontiguousarray(v, dtype=f32) for k, v in m.items()}


def kernel(s0, xs_h, uh, ys, ys_mask, emb, W_sa, b_sa, W_a1, b_a1,
           Wxz, Whz, Wcz, bz, Wxr, Whr, Wcr, br, Wxh, Whh, Wch, bh,
           W_ls, b_ls, W_ly, b_ly, W_lc, b_lc):
    from concourse.bass_utils import run_bass_kernel_spmd

    f = lambda a: np.asarray(a, dtype=np.float32)
    s0, xs_h, uh, ys_mask, emb = f(s0), f(xs_h), f(uh), f(ys_mask), f(emb)
    W_sa, b_sa, W_a1 = f(W_sa), f(b_sa), f(W_a1)
    Wxz, Whz, Wcz, bz = f(Wxz), f(Whz), f(Wcz), f(bz)
    Wxr, Whr, Wcr, br = f(Wxr), f(Whr), f(Wcr), f(br)
    Wxh, Whh, Wch, bh = f(Wxh), f(Whh), f(Wch), f(bh)
    W_ls, b_ls, W_ly, b_ly, W_lc, b_lc = f(W_ls), f(b_ls), f(W_ly), f(b_ly), f(W_lc), f(b_lc)

    ys_e = emb[np.asarray(ys, dtype=np.int64)]

    if "nc" not in _CACHE:
        _CACHE["nc"] = _build()
    nc = _CACHE["nc"]

    cc = {}
    in_maps = [_prep_core(c, s0, xs_h, uh, ys_e, W_sa, b_sa, W_a1,
                          Wxz, Whz, Wcz, bz, Wxr, Whr, Wcr, br,
                          Wxh, Whh, Wch, bh, W_ls, b_ls, W_ly, b_ly,
                          W_lc, b_lc, cc)
               for c in range(NCORES)]

    res = run_bass_kernel_spmd(nc, in_maps, list(range(NCORES)))
    _CACHE["last_res"] = res
    full = np.concatenate([r["out"] for r in res.results], axis=1)
    return (full * ys_mask[:, :, None]).astype(np.float32)
